# revision 4
# baseline (speedup 1.0000x reference)
"""Trainium2 Bass kernel for nn_LocallyDense: gather -> 16 group-GEMMs -> Conv1D(k=32) -> LeakyReLU.

Strategy: data-parallel over batch (32 -> 4 per core on 8 cores).
Host staging: apply the idx permutation + transpose while sharding (pure data
movement), so the device sees dense GEMMs only:
  stage 1: h[b] = x_perm[b] @ W[g] per group, computed as hT[d, (b,pos)]
  stage 2: y[b] = LeakyReLU(conv1d(h[b])) as a K=8192 GEMM accumulated in PSUM.
All matmuls in float32r (full-rate at moving-dim >= 256, ~1.5e-4 rel err).
Device output layout y[b, o, t]; host transposes back to [b, t, o].
"""
import numpy as np

import concourse.bass as bass
import concourse.mybir as mybir
import concourse.tile as tile
from concourse.alu_op_type import AluOpType
from concourse import bacc
from concourse.bass_utils import run_bass_kernel_spmd

B, N, F, G, S, D = 32, 1024, 512, 16, 64, 256
KC, O = 32, 512            # conv kernel taps, conv out channels
T = N - KC + 1             # 993 valid conv outputs
NCORES = 8
BPC = B // NCORES          # batches per core
NEG_SLOPE = 0.2
F32 = mybir.dt.float32
F32R = mybir.dt.float32r

TRACE = False              # test.py flips this to get a profile
_cache = {}


def _build():
    nc = bacc.Bacc("TRN2", target_bir_lowering=False, debug=False,
                   num_devices=NCORES)
    xpt_d = nc.dram_tensor("xpt", [BPC, F, N], F32, kind="ExternalInput").ap()
    w_d = nc.dram_tensor("w", [G, F, D], F32, kind="ExternalInput").ap()
    b_d = nc.dram_tensor("b", [G, D], F32, kind="ExternalInput").ap()
    cw_d = nc.dram_tensor("cw", [4, KC * 2, 128, 128], F32,
                          kind="ExternalInput").ap()
    cb_d = nc.dram_tensor("cb", [O], F32, kind="ExternalOutput"
                          if False else "ExternalInput").ap()
    y_d = nc.dram_tensor("y", [BPC, O, T], F32, kind="ExternalOutput").ap()

    FKT = F // 128           # 4 k-tiles over F
    KK = KC * 2              # 64 k-chunks over (tap, d-half)
    with tile.TileContext(nc) as tc:
        with tc.tile_pool(name="xpt", bufs=4) as p_xpt, \
             tc.tile_pool(name="wg", bufs=2) as p_w, \
             tc.tile_pool(name="ht", bufs=1) as p_ht, \
             tc.tile_pool(name="bias", bufs=1) as p_bias, \
             tc.tile_pool(name="cw", bufs=2) as p_cw, \
             tc.tile_pool(name="yout", bufs=4) as p_out:

            # biases: b[g, m*128+p] -> b_sb[p, g*2+m]; conv_b[m*128+p] -> cb_sb[p, m]
            b_sb = p_bias.tile([128, G * 2], F32)
            nc.sync.dma_start(b_sb[:], b_d.rearrange("g (m p) -> p (g m)", p=128))
            cb_sb = p_bias.tile([128, 4], F32)
            nc.sync.dma_start(cb_sb[:], cb_d.rearrange("(m p) -> p m", p=128))

            # x permuted+transposed: per f-ktile a [128, BPC*N] tile, b-major cols
            xpt_sb = []
            for kt in range(FKT):
                t = p_xpt.tile([128, BPC * N], F32R, tag="xpt")
                nc.sync.dma_start(
                    t[:].rearrange("p (b t) -> p b t", b=BPC),
                    xpt_d[:, kt * 128:(kt + 1) * 128, :]
                        .bitcast(F32R).rearrange("b p t -> p b t"))
                xpt_sb.append(t)

            # conv weights per o-tile m: [128, KK*128], chunk k holds lhsT
            cw_sb = []
            for m in range(4):
                t = p_cw.tile([128, KK * 128], F32R, tag="cw")
                nc.sync.dma_start(
                    t[:].rearrange("p (k o) -> p k o", k=KK),
                    cw_d[m].bitcast(F32R).rearrange("k p o -> p k o"))
                cw_sb.append(t)

            # hT[m]: [128, BPC*N] float32r, d-half m on partitions
            ht_sb = [p_ht.tile([128, BPC * N], F32R, tag=f"ht{m}", name=f"ht{m}")
                     for m in range(2)]

            # ---------------- stage 1: group GEMMs ----------------
            with tc.tile_pool(name="ps1", bufs=4, space="PSUM") as p_ps1:
                for g in range(G):
                    w_sb = p_w.tile([128, FKT * D], F32R, tag="wg")
                    nc.sync.dma_start(
                        w_sb[:].rearrange("p (kt d) -> p kt d", kt=FKT),
                        w_d[g].bitcast(F32R).rearrange("(kt p) d -> p kt d", p=128))
                    for m in range(2):
                        ps = p_ps1.tile([128, BPC * S], F32, tag="ps1")
                        for kt in range(FKT):
                            rhs = xpt_sb[kt][:].rearrange(
                                "p (b t) -> p b t", b=BPC)[:, :, g * S:(g + 1) * S]
                            nc.tensor.matmul(
                                ps[:], w_sb[:, kt * D + m * 128: kt * D + (m + 1) * 128],
                                rhs, start=(kt == 0), stop=(kt == FKT - 1))
                        dest = ht_sb[m][:].rearrange(
                            "p (b t) -> p b t", b=BPC)[:, :, g * S:(g + 1) * S]
                        # h = psum + bias[g, m-half] (copy + rounds to f32r)
                        nc.vector.tensor_scalar_add(
                            dest, ps[:], b_sb[:, g * 2 + m: g * 2 + m + 1])

            # ---------------- stage 2: conv as GEMM ----------------
            ntiles = []
            for bb in range(BPC):
                ntiles.append((bb, 0, 512))
                # fp32r ISA requires even element counts; overlap col 511
                ntiles.append((bb, 511, 482))
            with tc.tile_pool(name="ps2", bufs=8, space="PSUM") as p_ps2:
                for m in range(4):
                    pss = [p_ps2.tile([128, 512], F32, tag="ps2", name=f"ps2_{m}_{j}")
                           for j in range(len(ntiles))]
                    for k in range(KK):
                        tap, dh = k // 2, k % 2
                        lhsT = cw_sb[m][:, k * 128:(k + 1) * 128]
                        for j, (bb, t0, nn) in enumerate(ntiles):
                            rhs = ht_sb[dh][:, bb * N + tap + t0:
                                            bb * N + tap + t0 + nn]
                            nc.tensor.matmul(pss[j][:, :nn], lhsT, rhs,
                                             start=(k == 0), stop=(k == KK - 1))
                    for j, (bb, t0, nn) in enumerate(ntiles):
                        y_sb = p_out.tile([128, 512], F32, tag="yout")
                        nc.vector.tensor_scalar_add(
                            y_sb[:, :nn], pss[j][:, :nn], cb_sb[:, m:m + 1])
                        nc.vector.scalar_tensor_tensor(
                            y_sb[:, :nn], y_sb[:, :nn], NEG_SLOPE, y_sb[:, :nn],
                            AluOpType.mult, AluOpType.max)
                        nc.sync.dma_start(
                            y_d[bb, m * 128:(m + 1) * 128, t0:t0 + nn],
                            y_sb[:, :nn])
    nc.compile()
    return nc


def kernel(x, idx, W, b, conv_w, conv_b):
    x = np.asarray(x); idx = np.asarray(idx); W = np.asarray(W)
    b = np.asarray(b); conv_w = np.asarray(conv_w); conv_b = np.asarray(conv_b)
    if "nc" not in _cache:
        _cache["nc"] = _build()
    nc = _cache["nc"]

    idx_flat = idx.reshape(-1).astype(np.int64)
    # permute + transpose: xpt[b, f, p] = x[b, idx_flat[p], f]
    xpt = np.ascontiguousarray(x[:, idx_flat, :].transpose(0, 2, 1))
    # conv_w[tap, d, o] -> cw[m, (tap, dh), p, o]
    cw = np.ascontiguousarray(
        conv_w.reshape(KC, 2, 128, 4, 128).transpose(3, 0, 1, 2, 4)
    ).reshape(4, KC * 2, 128, 128)
    W_c = np.ascontiguousarray(W)
    b_c = np.ascontiguousarray(b)
    cb_c = np.ascontiguousarray(conv_b)

    in_maps = []
    for c in range(NCORES):
        in_maps.append({
            "xpt": xpt[c * BPC:(c + 1) * BPC],
            "w": W_c, "b": b_c, "cw": cw, "cb": cb_c,
        })
    res = run_bass_kernel_spmd(nc, in_maps, core_ids=list(range(NCORES)),
                               trace=TRACE)
    if TRACE and res.exec_time_ns is not None:
        print(f"HW exec time: {res.exec_time_ns} ns")
        if res.instructions_and_trace is not None:
            print("trace:", res.instructions_and_trace[1])
    y = np.concatenate([r["y"] for r in res.results], axis=0)  # [B, O, T]
    return np.ascontiguousarray(y.transpose(0, 2, 1)).astype(np.float32)
